# revision 7
# baseline (speedup 1.0000x reference)
"""Trainium2 Bass kernel for a post-LN transformer block (MHA + FFN).

Contract: kernel(**inputs) takes the FULL unsharded inputs (as produced by
the problem's setup_inputs) and returns the FULL output [2, 2048, 1024].

Sharding: token-parallel across 8 cores. Core c handles 512 tokens of
batch c//4. K^T and V are all-gathered in TWO coalesced AllGathers
(head-pairs 0-3 and 4-7) within each 4-core replica group, fired as early
as possible; everything else is collective-free.

Softmax exp is split across engines: even heads run on ScalarE (table
exp, scale=8 since scores arrive pre-divided by 8), odd heads run on
VectorE via a custom 8-stage DVE op computing cubic(s/8)^8 (the global
scale constant of the cubic cancels in the per-head softmax normalize).

Matmuls run in bf16 (fp32 PSUM accumulation).
"""
import sys

for _p in ('/opt/trn_rl_repo', '/opt/pypackages'):
    if _p not in sys.path:
        sys.path.insert(0, _p)

import numpy as np
import ml_dtypes
import concourse.bass as bass
import concourse.tile as tile
from concourse import bacc, mybir
from concourse.bass import ts
from concourse.masks import make_identity
from contextlib import ExitStack

# ---- profiling shim (enables trace=True under axon; harmless if unused) ----
def _install_prof_shim():
    import types
    if 'antenv.axon_hooks' in sys.modules:
        return
    try:
        import trn_agent_boot.trn_boot as tb
        hook = tb._ntff_profile_via_ctypes('/opt/axon/libaxon_pjrt.so')
    except Exception:
        hook = None
    mod = types.ModuleType('antenv.axon_hooks')
    mod.get_axon_ntff_profile_hook = lambda: hook
    mod.set_axon_ntff_profile_hook = lambda h: None
    sys.modules['antenv.axon_hooks'] = mod

_install_prof_shim()

from concourse.bass_utils import run_bass_kernel_spmd  # noqa: E402

# ---- custom DVE op: exp(8u) ~ ((u+A)*((u+B)^2+C))^8 (global scale drops
# out in the softmax normalize). Registered via the documented dve_ops
# extension point. 8 ALU stages = the v3 pipeline exactly.
from concourse import dve_ops as _dve_ops  # noqa: E402
from concourse.dve_spec import Spec as _Spec, Src0 as _Src0, C0 as _C0, \
    C1 as _C1, C2 as _C2, sq as _sq, lower as _dve_lower, \
    _has_src1 as _dve_has_src1  # noqa: E402
from concourse.dve_uop import DveOpSpec as _DveOpSpec  # noqa: E402

EXP_A = 1.6582839286327529
EXP_B = 0.7283352027775907
EXP_C = 3.124158585517612


def _exp8_ref(in0, in1, s0, s1, imm2):
    x = in0.astype(np.float32)
    q = (x + s0) * ((x + s1) ** 2 + imm2)
    return ((q * q) ** 2) ** 2


def _register_exp8():
    if 'EXP8_CUBIC_ANT' in _dve_ops._SUB_OPCODE_FOR_NAME:
        return next(o for o in _dve_ops.OPS if o.name == 'EXP8_CUBIC_ANT')
    body = _sq(_sq(_sq((_Src0 + _C0) * (_sq(_Src0 + _C1) + _C2))))
    spec = _Spec(body=body, reference=_exp8_ref)
    op = _dve_ops.DveOp('EXP8_CUBIC_ANT', spec, subdim=False, uops_sha={})
    row = max(_dve_ops._SUB_OPCODE_FOR_NAME.values()) + 1
    _dve_ops.OPS.append(op)
    _dve_ops.CUSTOM_DVE_SPECS[op.name] = op.spec
    _dve_ops._SUB_OPCODE_FOR_NAME[op.name] = row
    for ver in ('v3', 'v4'):
        compiled = _DveOpSpec(name=op.name, opcode=row,
                              uops=_dve_lower(spec, ver=ver),
                              rd1_en=_dve_has_src1(spec))
        op.uops_sha[ver] = compiled.sha(ver)
    return op


EXP8_OP = _register_exp8()

B, S, H, NH, HD = 2, 2048, 1024, 16, 64
P = 128
NCORES = 8
GSIZE = 4                    # replica-group size (cores per batch)
TQ = S // GSIZE              # tokens per core = 512
FT = H // P                  # feature tiles = 8
MT = TQ // P                 # token tiles per core = 4
EPS = 1e-5
RG = [[0, 1, 2, 3], [4, 5, 6, 7]]
KVT = 2 * P * TQ             # elems per head-pair block (K^T tile + V tile)
HT = 4                       # head-pairs per AllGather (2 AllGathers total)
WC = 512                     # weight chunk (output cols per streamed tile)

f32 = mybir.dt.float32
bf16 = mybir.dt.bfloat16
AF = mybir.ActivationFunctionType
ALU = mybir.AluOpType


def build_kernel():
    nc = bacc.Bacc("TRN2", target_bir_lowering=False, debug=False,
                   num_devices=NCORES)

    def din(name, shape, dt=f32):
        return nc.dram_tensor(name, shape, dt, kind="ExternalInput").ap()

    # inputs (per-core values supplied via in_maps). Weights are host-side
    # re-laid-out as [P, nchunk, FT, WC] so each chunk is one contiguous
    # per-partition DMA.
    xtp = din("xtp", [P, FT * TQ], bf16)    # x slice ^T, p-major
    xnb = din("xnb", [TQ, H])               # x slice natural + bo pre-added
    wqp = din("wqp", [P, H // WC, FT, WC], bf16)
    wkp = din("wkp", [P, H // WC, FT, WC], bf16)   # scaled by 1/(8*sqrt(hd))
    wvp = din("wvp", [P, H // WC, FT, WC], bf16)
    wop = din("wop", [P, H // WC, FT, WC], bf16)
    w1p = din("w1p", [P, H // WC, FT, WC], bf16)
    w2p = din("w2p", [P, H // WC, FT, WC], bf16)
    bqp = din("bqp", [P, FT])               # bq as [part, tile]
    bkp = din("bkp", [P, FT])               # bk * 1/(8*sqrt(hd))
    b1p = din("b1p", [P, FT])               # b1 + W1 @ be1
    bvB = din("bvB", [P, H])                # broadcast rows
    b2B = din("b2B", [P, H])                # b2 + be1
    g1B = din("g1B", [P, H])
    g2B = din("g2B", [P, H])
    be2B = din("be2B", [P, H])
    onesc = din("onesc", [P, MT], bf16)
    nri = din("nri", [P, 2], mybir.dt.int32)     # [1, -1] int scalars
    nrm = din("nrm", [P, MT], mybir.dt.int32)    # magic+1
    y = nc.dram_tensor("y", [TQ, H], f32, kind="ExternalOutput").ap()

    bounce = [nc.dram_tensor(f"bounce{g}", [HT, KVT], bf16).ap()
              for g in range(2)]
    agout = [nc.dram_tensor(f"agout{g}", [GSIZE, HT, KVT], bf16).ap()
             for g in range(2)]

    with tile.TileContext(nc) as tc, ExitStack() as ctx:
        # ---------------- persistent pools ----------------
        const = ctx.enter_context(tc.tile_pool(name="const", bufs=1))
        acts = ctx.enter_context(tc.tile_pool(name="acts", bufs=1))
        wpool = ctx.enter_context(tc.tile_pool(name="w", bufs=3))

        # constants
        bq_s = const.tile([P, FT], f32)
        nc.sync.dma_start(bq_s[:], bqp)
        bk_s = const.tile([P, FT], f32)
        nc.sync.dma_start(bk_s[:], bkp)
        b1_s = const.tile([P, FT], f32)
        nc.sync.dma_start(b1_s[:], b1p)
        bvB_s = const.tile([P, H], f32)
        nc.sync.dma_start(bvB_s[:], bvB)
        b2B_s = const.tile([P, H], f32)
        g1B_s = const.tile([P, H], f32)
        g2B_s = const.tile([P, H], f32)
        be2B_s = const.tile([P, H], f32)
        ones_s = const.tile([P, MT], bf16)
        nc.sync.dma_start(ones_s[:], onesc)
        nri_s = const.tile([P, 2], mybir.dt.int32)
        nc.sync.dma_start(nri_s[:], nri)
        nrm_s = const.tile([P, MT], mybir.dt.int32)
        nc.sync.dma_start(nrm_s[:], nrm)
        eps_s = const.tile([P, 1], f32)
        nc.vector.memset(eps_s[:], EPS)
        warm_s = const.tile([P, 1], f32)
        nc.scalar.activation(warm_s[:], eps_s[:], AF.Exp)
        ident = const.tile([P, P], f32)
        make_identity(nc, ident)

        # resident activations (lifetimes span phases)
        xnb_s = acts.tile([P, MT, H], f32)       # natural x + bo
        qt_s = acts.tile([P, FT, TQ], bf16)      # Q^T
        ctxT_s = acts.tile([P, FT, TQ], bf16)    # attention ctx^T (normalized)
        ln1_s = acts.tile([P, MT, H], f32)       # LN1 out (gamma applied)
        ln1b2_s = acts.tile([P, MT, H], f32)     # ln1 + b2 (residual for fc2)
        ln1T_s = acts.tile([P, FT, TQ], bf16)    # LN1 transposed
        hT_s = acts.tile([P, FT, TQ], bf16)      # relu(fc1), transposed

        # ------------- phase A: K, V projections + 2 AllGathers ----------
        with tc.tile_pool(name="xa", bufs=1) as xa, \
             tc.tile_pool(name="psA1", bufs=2, space="PSUM") as psA:
            kt_s = xa.tile([P, FT, TQ], bf16)    # K^T (scaled), all 8 pairs
            v_s = xa.tile([P, MT, H], bf16)      # V natural
            xt_s = xa.tile([P, FT, TQ], bf16)
            nc.sync.dma_start(xt_s[:], xtp.rearrange("p (t n) -> p t n", n=TQ))
            for g in range(2):      # AllGather group g: head-pairs 4g..4g+3
                wk_c = wpool.tile([P, FT, WC], bf16, tag="w")
                nc.sync.dma_start(wk_c[:], wkp[:, g])
                for mi in range(WC // P):       # K^T tiles (t = 4g+mi)
                    t = g * HT + mi
                    ps = psA.tile([P, TQ], f32, tag="psA")
                    for kt in range(FT):
                        nc.tensor.matmul(ps[:], wk_c[:, kt, ts(mi, P)],
                                         xt_s[:, kt, :],
                                         start=(kt == 0), stop=(kt == FT - 1))
                    nc.scalar.activation(kt_s[:, t, :], ps[:], AF.Identity,
                                         bias=bk_s[:, t:t + 1])
                wv_c = wpool.tile([P, FT, WC], bf16, tag="w")
                nc.sync.dma_start(wv_c[:], wvp[:, g])
                for m in range(MT):             # V chunk (cols 512g..512g+511)
                    ps = psA.tile([P, WC], f32, tag="psA")
                    for kt in range(FT):
                        nc.tensor.matmul(ps[:], xt_s[:, kt, ts(m, P)],
                                         wv_c[:, kt, :],
                                         start=(kt == 0), stop=(kt == FT - 1))
                    nc.vector.tensor_tensor(
                        out=v_s[:, m, ts(g, WC)], in0=ps[:],
                        in1=bvB_s[:, ts(g, WC)], op=ALU.add)
                # bounce + one AllGather for the whole 4-pair group
                nc.sync.dma_start(
                    bounce[g][:, 0:P * TQ].rearrange("t (p n) -> p t n", p=P),
                    kt_s[:, g * HT:(g + 1) * HT, :])
                for tl in range(HT):
                    nc.sync.dma_start(
                        bounce[g][tl, P * TQ:KVT]
                        .rearrange("(m p f) -> p m f", p=P, f=P),
                        v_s[:, :, ts(g * HT + tl, P)])
                nc.gpsimd.collective_compute(
                    "AllGather", ALU.bypass, replica_groups=RG,
                    ins=[bounce[g]], outs=[agout[g]])
            # Q^T projection overlaps with the AllGathers
            for g in range(2):
                wq_c = wpool.tile([P, FT, WC], bf16, tag="w")
                nc.sync.dma_start(wq_c[:], wqp[:, g])
                for mi in range(WC // P):
                    t = g * HT + mi
                    ps = psA.tile([P, TQ], f32, tag="psA")
                    for kt in range(FT):
                        nc.tensor.matmul(ps[:], wq_c[:, kt, ts(mi, P)],
                                         xt_s[:, kt, :],
                                         start=(kt == 0), stop=(kt == FT - 1))
                    nc.scalar.activation(qt_s[:, t, :], ps[:], AF.Identity,
                                         bias=bq_s[:, t:t + 1])

        # deferred constant loads (not needed until phases B-F)
        nc.gpsimd.dma_start(xnb_s[:], xnb.rearrange("(m p) f -> p m f", p=P))
        nc.gpsimd.dma_start(b2B_s[:], b2B)
        nc.gpsimd.dma_start(g1B_s[:], g1B)
        nc.gpsimd.dma_start(g2B_s[:], g2B)
        nc.gpsimd.dma_start(be2B_s[:], be2B)

        # ---------------- phase B: attention ----------------
        with tc.tile_pool(name="kvt", bufs=6) as kvt, \
             tc.tile_pool(name="esb", bufs=4) as esb, \
             tc.tile_pool(name="psS", bufs=2, space="PSUM") as psS, \
             tc.tile_pool(name="psC", bufs=2, space="PSUM") as psC, \
             tc.tile_pool(name="rec", bufs=1) as rec:
            for t in range(FT):          # head pair (2t, 2t+1)
                g, tl = t // HT, t % HT
                ps_c0 = psC.tile([P, TQ], f32, tag="c0")   # rows0-63 ctx, 64 sums
                ps_c1 = psC.tile([P, TQ], f32, tag="c1")
                first = True
                for rb in range(GSIZE):
                    ktile = kvt.tile([P, TQ], bf16, tag="k")
                    nc.sync.dma_start(
                        ktile[:],
                        agout[g][rb, tl, 0:P * TQ].rearrange("(p n) -> p n", p=P))
                    vbase = agout[g][rb, tl, P * TQ:KVT] \
                        .rearrange("(m p f) -> p m f", p=P, f=P)
                    vt0 = kvt.tile([P, MT, HD + 1], bf16, tag="v0")
                    nc.sync.dma_start(vt0[:, :, 0:HD], vbase[:, :, 0:HD])
                    nc.vector.tensor_copy(vt0[:, :, HD:HD + 1],
                                          ones_s.unsqueeze(2))
                    vt1 = kvt.tile([P, MT, HD + 1], bf16, tag="v1")
                    nc.sync.dma_start(vt1[:, :, 0:HD], vbase[:, :, HD:P])
                    nc.vector.tensor_copy(vt1[:, :, HD:HD + 1],
                                          ones_s.unsqueeze(2))
                    for sj in range(MT):
                        last = (rb == GSIZE - 1 and sj == MT - 1)
                        ps = psS.tile([P, 2, TQ], f32, tag="s")
                        nc.tensor.matmul(ps[:, 0, :],
                                         ktile[0:HD, ts(sj, P)],
                                         qt_s[0:HD, t, :],
                                         start=True, stop=True)
                        nc.tensor.matmul(ps[:, 1, :],
                                         ktile[HD:P, ts(sj, P)],
                                         qt_s[HD:P, t, :],
                                         start=True, stop=True)
                        e = esb.tile([P, 2, TQ], bf16, tag="e")
                        # head 2t: ScalarE table exp (scores arrive as s/8)
                        nc.scalar.activation(e[:, 0, :], ps[:, 0, :], AF.Exp,
                                             scale=8.0)
                        # head 2t+1: VectorE cubic^8 exp
                        nc.vector._custom_dve(
                            EXP8_OP, out=e[:, 1, :], in0=ps[:, 1, :],
                            s0=EXP_A, s1=EXP_B, imm2=EXP_C)
                        nc.tensor.matmul(ps_c0[0:HD + 1, :], vt0[:, sj, :],
                                         e[:, 0, :], start=first, stop=last)
                        nc.tensor.matmul(ps_c1[0:HD + 1, :], vt1[:, sj, :],
                                         e[:, 1, :], start=first, stop=last)
                        first = False
                # normalize: rows 0-63 / row 64
                sr0 = rec.tile([HD + 1, TQ], f32, tag="sr0")
                nc.vector.tensor_copy(sr0[HD:HD + 1, :], ps_c0[HD:HD + 1, :])
                rr0 = rec.tile([1, TQ], f32, tag="rr0")
                nc.gpsimd.dma_start(rr0[:], sr0[HD:HD + 1, :])
                nc.vector.reciprocal_approx_fast(rr0[:], rr0[:])
                rb0 = rec.tile([HD, TQ], f32, tag="rb0")
                nc.gpsimd.partition_broadcast(rb0[:], rr0[:])
                nc.vector.tensor_tensor(out=ctxT_s[0:HD, t, :], in0=ps_c0[0:HD, :],
                                        in1=rb0[:], op=ALU.mult)
                sr1 = rec.tile([HD + 1, TQ], f32, tag="sr1")
                nc.vector.tensor_copy(sr1[HD:HD + 1, :], ps_c1[HD:HD + 1, :])
                rr1 = rec.tile([1, TQ], f32, tag="rr1")
                nc.gpsimd.dma_start(rr1[:], sr1[HD:HD + 1, :])
                nc.vector.reciprocal_approx_fast(rr1[:], rr1[:])
                rb1 = rec.tile([HD, TQ], f32, tag="rb1")
                nc.gpsimd.partition_broadcast(rb1[:], rr1[:])
                c1t = rec.tile([HD, TQ], bf16, tag="c1t")
                nc.vector.tensor_tensor(out=c1t[:], in0=ps_c1[0:HD, :],
                                        in1=rb1[:], op=ALU.mult)
                nc.gpsimd.dma_start(ctxT_s[HD:P, t, :], c1t[:])

        # ---------------- LN helper ----------------
        def ln_stats(src_s, stat_pool):
            # src_s: [P, MT, H]; stats over free dim H. Returns (mv, rstd, nmr)
            mv = stat_pool.tile([P, MT, 2], f32, tag="mv")
            for m in range(MT):
                stats = stat_pool.tile([P, 2, 6], f32, tag="bst")
                for sg in range(2):
                    nc.vector.bn_stats(out=stats[:, sg, :],
                                       in_=src_s[:, m, ts(sg, H // 2)])
                nc.vector.bn_aggr(out=mv[:, m, :], in_=stats[:])
            # rstd = rsqrt(var + eps) via int-magic seed + 3 Newton steps
            ve = stat_pool.tile([P, MT], f32, tag="ve")
            nc.vector.tensor_scalar(out=ve[:], in0=mv[:, :, 1], scalar1=EPS,
                                    scalar2=None, op0=ALU.add)
            it = stat_pool.tile([P, MT], mybir.dt.int32, tag="it")
            nc.vector.tensor_scalar(out=it[:], in0=ve.bitcast(mybir.dt.int32),
                                    scalar1=nri_s[:, 0:1], scalar2=None,
                                    op0=ALU.logical_shift_right)
            nc.vector.tensor_scalar(out=it[:], in0=it[:], scalar1=nri_s[:, 1:2],
                                    scalar2=None, op0=ALU.bitwise_xor)
            nc.vector.tensor_tensor(out=it[:], in0=it[:], in1=nrm_s[:],
                                    op=ALU.add)
            rstd = it.bitcast(f32)
            nrt = stat_pool.tile([P, MT], f32, tag="nrt")
            for _ in range(3):
                nc.vector.tensor_tensor(out=nrt[:], in0=rstd, in1=rstd,
                                        op=ALU.mult)
                nc.vector.tensor_tensor(out=nrt[:], in0=nrt[:], in1=ve[:],
                                        op=ALU.mult)
                nc.vector.tensor_scalar(out=nrt[:], in0=nrt[:], scalar1=-0.5,
                                        scalar2=1.5, op0=ALU.mult, op1=ALU.add)
                nc.vector.tensor_tensor(out=rstd, in0=rstd, in1=nrt[:],
                                        op=ALU.mult)
            # nmr = -mu * rstd  (per-token bias for the fused affine apply)
            nmr = stat_pool.tile([P, MT], f32, tag="nmr")
            nc.vector.tensor_tensor(out=nmr[:], in0=mv[:, :, 0], in1=rstd,
                                    op=ALU.mult)
            nc.vector.tensor_scalar(out=nmr[:], in0=nmr[:], scalar1=-1.0,
                                    scalar2=None, op0=ALU.mult)
            return rstd, nmr

        from concourse.dve_ops import AFFINE_MUL_REDUCE

        # ---------------- phases C-F ----------------
        with tc.tile_pool(name="lnp", bufs=2) as lnp, \
             tc.tile_pool(name="wmm", bufs=1) as wmmpool, \
             tc.tile_pool(name="psA2", bufs=2, space="PSUM") as psA, \
             tc.tile_pool(name="pst", bufs=2, space="PSUM") as pst:
            # C: Wo + residual -> t1, then LN1 (fused gamma apply)
            t1_s = acts.tile([P, MT, H], f32, tag="tres")
            wo_cs = []
            for g in range(2):
                w_s = wmmpool.tile([P, FT, WC], bf16, tag=f"wmm{g}")
                nc.sync.dma_start(w_s[:], wop[:, g])
                wo_cs.append(w_s)
            for m in range(MT):
                for g in range(2):
                    ps = psA.tile([P, WC], f32, tag="psN")
                    for kt in range(FT):
                        nc.tensor.matmul(ps[:], ctxT_s[:, kt, ts(m, P)],
                                         wo_cs[g][:, kt, :],
                                         start=(kt == 0), stop=(kt == FT - 1))
                    nc.vector.tensor_tensor(
                        out=t1_s[:, m, ts(g, WC)], in0=ps[:],
                        in1=xnb_s[:, m, ts(g, WC)], op=ALU.add)
            rstd1, nmr1 = ln_stats(t1_s, lnp)
            for m in range(MT):
                nc.vector._custom_dve(
                    AFFINE_MUL_REDUCE, out=ln1_s[:, m, :], in0=t1_s[:, m, :],
                    in1=g1B_s[:], s0=rstd1[:, m:m + 1], s1=nmr1[:, m:m + 1])
                # residual for fc2 (b2 includes the folded be1)
                nc.gpsimd.tensor_tensor(out=ln1b2_s[:, m, :],
                                        in0=ln1_s[:, m, :],
                                        in1=b2B_s[:], op=ALU.add)

            # D: transpose ln1 -> ln1T (PE transpose, ScalarE eviction)
            for ft in range(FT):
                for m in range(MT):
                    pt = pst.tile([P, P], f32, tag="pt")
                    nc.tensor.transpose(pt[:], ln1_s[:, m, ts(ft, P)], ident[:])
                    nc.scalar.copy(ln1T_s[:, ft, ts(m, P)], pt[:])

            # E: fc1 + relu on ScalarE (transposed out)
            for g in range(2):
                w_s = wpool.tile([P, FT, WC], bf16, tag="w")
                nc.sync.dma_start(w_s[:], w1p[:, g])
                for mi in range(WC // P):
                    mt_i = g * HT + mi
                    ps = psA.tile([P, TQ], f32, tag="psT")
                    for kt in range(FT):
                        nc.tensor.matmul(ps[:], w_s[:, kt, ts(mi, P)],
                                         ln1T_s[:, kt, :],
                                         start=(kt == 0), stop=(kt == FT - 1))
                    nc.scalar.activation(hT_s[:, mt_i, :], ps[:], AF.Relu,
                                         bias=b1_s[:, mt_i:mt_i + 1])

            # F: fc2 + residual + LN2 + out
            t2_s = acts.tile([P, MT, H], f32, tag="tres")
            w2_cs = []
            for g in range(2):
                w_s = wmmpool.tile([P, FT, WC], bf16, tag=f"wmm{g}")
                nc.sync.dma_start(w_s[:], w2p[:, g])
                w2_cs.append(w_s)
            for m in range(MT):
                for g in range(2):
                    ps = psA.tile([P, WC], f32, tag="psN")
                    for kt in range(FT):
                        nc.tensor.matmul(ps[:], hT_s[:, kt, ts(m, P)],
                                         w2_cs[g][:, kt, :],
                                         start=(kt == 0), stop=(kt == FT - 1))
                    nc.vector.tensor_tensor(
                        out=t2_s[:, m, ts(g, WC)], in0=ps[:],
                        in1=ln1b2_s[:, m, ts(g, WC)], op=ALU.add)
            rstd2, nmr2 = ln_stats(t2_s, lnp)
            y_r = y.rearrange("(m p) f -> p m f", p=P)
            for m in range(MT):
                nc.vector._custom_dve(
                    AFFINE_MUL_REDUCE, out=t2_s[:, m, :], in0=t2_s[:, m, :],
                    in1=g2B_s[:], s0=rstd2[:, m:m + 1], s1=nmr2[:, m:m + 1])
                nc.vector.tensor_tensor(out=t2_s[:, m, :], in0=t2_s[:, m, :],
                                        in1=be2B_s[:], op=ALU.add)
                nc.sync.dma_start(y_r[:, m, :], t2_s[:, m, :])

    nc.compile()
    return nc


_NC_CACHE = {}


def _get_nc():
    if 'nc' not in _NC_CACHE:
        _NC_CACHE['nc'] = build_kernel()
    return _NC_CACHE['nc']


def _bf(a):
    return np.ascontiguousarray(np.asarray(a, np.float32)).astype(
        ml_dtypes.bfloat16)


def _wchunks(WT):
    # [H_in, H_out] -> [P, H//WC, FT, WC]: w[p, h, t, c] = WT[t*128+p, h*WC+c]
    w = np.asarray(WT, np.float32).reshape(FT, P, H // WC, WC)
    return _bf(np.ascontiguousarray(w.transpose(1, 2, 0, 3)))


def make_in_maps(x, Wq, bq, Wk, bk, Wv, bv, Wo, bo, W1, b1, W2, b2,
                 g1, be1, g2, be2):
    def pt(b):  # [H] -> [P, FT] partition-tiled
        return np.ascontiguousarray(np.asarray(b, np.float32).reshape(FT, P).T)

    def bc(v):  # [H] -> [P, H] broadcast
        return np.ascontiguousarray(
            np.broadcast_to(np.asarray(v, np.float32), (P, H)))

    scale = np.float32(1.0 / (8.0 * np.sqrt(HD)))
    W1f = np.asarray(W1, np.float32)
    b1f = np.asarray(b1, np.float32) + W1f @ np.asarray(be1, np.float32)
    b2f = np.asarray(b2, np.float32) + np.asarray(be1, np.float32)
    shared = {
        "wqp": _wchunks(np.asarray(Wq, np.float32).T),
        "wkp": _wchunks(np.asarray(Wk, np.float32).T * scale),
        "wvp": _wchunks(np.asarray(Wv, np.float32).T),
        "wop": _wchunks(np.asarray(Wo, np.float32).T),
        "w1p": _wchunks(W1f.T),
        "w2p": _wchunks(np.asarray(W2, np.float32).T),
        "bqp": pt(bq),
        "bkp": pt(np.asarray(bk, np.float32) * scale),
        "b1p": pt(b1f),
        "bvB": bc(bv), "b2B": bc(b2f),
        "g1B": bc(g1), "g2B": bc(g2), "be2B": bc(be2),
        "onesc": np.ones((P, MT), ml_dtypes.bfloat16),
        "nri": np.tile(np.array([[1, -1]], np.int32), (P, 1)),
        "nrm": np.full((P, MT), 0x5f3759df + 1, np.int32),
    }
    in_maps = []
    for c in range(NCORES):
        b, sl = c // GSIZE, (c % GSIZE) * TQ
        xs = np.asarray(x[b, sl:sl + TQ, :], np.float32)
        m = dict(shared)
        xT = _bf(xs.T)  # [H, TQ]
        m["xtp"] = np.ascontiguousarray(
            xT.reshape(FT, P, TQ).transpose(1, 0, 2).reshape(P, FT * TQ))
        m["xnb"] = np.ascontiguousarray(xs + np.asarray(bo, np.float32))
        in_maps.append(m)
    return in_maps


def kernel(x, Wq, bq, Wk, bk, Wv, bv, Wo, bo, W1, b1, W2, b2,
           g1, be1, g2, be2):
    x = np.asarray(x)
    nc = _get_nc()
    in_maps = make_in_maps(x, Wq, bq, Wk, bk, Wv, bv, Wo, bo,
                           W1, b1, W2, b2, g1, be1, g2, be2)
    res = run_bass_kernel_spmd(nc, in_maps, list(range(NCORES)))
    out = np.empty((B, S, H), np.float32)
    for c in range(NCORES):
        b, sl = c // GSIZE, (c % GSIZE) * TQ
        out[b, sl:sl + TQ, :] = res.results[c]["y"]
    return out


# revision 24
# speedup vs baseline: 1.1183x; 1.1183x over previous
"""Trainium2 Bass kernel for a post-LN transformer block (MHA + FFN).

Contract: kernel(**inputs) takes the FULL unsharded inputs (as produced by
the problem's setup_inputs) and returns the FULL output [2, 2048, 1024].

Sharding: token-parallel across 8 cores. Core c handles 512 tokens of
batch c//4. K^T and V are all-gathered in TWO coalesced AllGathers
(head-pairs 0-3 and 4-7) within each 4-core replica group, fired as early
as possible; everything else is collective-free.

Softmax exp is split across engines: even heads run on ScalarE (table
exp, scale=8 since scores arrive pre-divided by 8), odd heads run on
VectorE via a custom 8-stage DVE op computing cubic(s/8)^8 (the global
scale constant of the cubic cancels in the per-head softmax normalize).

Matmuls run in bf16 (fp32 PSUM accumulation).
"""
import sys

for _p in ('/opt/trn_rl_repo', '/opt/pypackages'):
    if _p not in sys.path:
        sys.path.insert(0, _p)

import numpy as np
import ml_dtypes
import concourse.bass as bass
import concourse.tile as tile
from concourse import bacc, mybir
from concourse.bass import ts
from concourse.masks import make_identity
from contextlib import ExitStack

# ---- profiling shim (enables trace=True under axon; harmless if unused) ----
def _install_prof_shim():
    import types
    if 'antenv.axon_hooks' in sys.modules:
        return
    try:
        import trn_agent_boot.trn_boot as tb
        hook = tb._ntff_profile_via_ctypes('/opt/axon/libaxon_pjrt.so')
    except Exception:
        hook = None
    mod = types.ModuleType('antenv.axon_hooks')
    mod.get_axon_ntff_profile_hook = lambda: hook
    mod.set_axon_ntff_profile_hook = lambda h: None
    sys.modules['antenv.axon_hooks'] = mod

_install_prof_shim()

from concourse.bass_utils import run_bass_kernel_spmd  # noqa: E402

# ---- custom DVE op: exp(8u) ~ ((u+A)*((u+B)^2+C))^8 (global scale drops
# out in the softmax normalize). Registered via the documented dve_ops
# extension point. 8 ALU stages = the v3 pipeline exactly.
from concourse import dve_ops as _dve_ops  # noqa: E402
from concourse.dve_spec import Spec as _Spec, Src0 as _Src0, C0 as _C0, \
    C1 as _C1, C2 as _C2, sq as _sq, lower as _dve_lower, \
    _has_src1 as _dve_has_src1  # noqa: E402
from concourse.dve_uop import DveOpSpec as _DveOpSpec  # noqa: E402

EXP_A = 1.6582839286327529
EXP_B = 0.7283352027775907
EXP_C = 3.124158585517612


def _exp8_ref(in0, in1, s0, s1, imm2):
    x = in0.astype(np.float32)
    q = (x + s0) * ((x + s1) ** 2 + imm2)
    return ((q * q) ** 2) ** 2


def _register_exp8():
    if 'EXP8_CUBIC_ANT' in _dve_ops._SUB_OPCODE_FOR_NAME:
        return next(o for o in _dve_ops.OPS if o.name == 'EXP8_CUBIC_ANT')
    body = _sq(_sq(_sq((_Src0 + _C0) * (_sq(_Src0 + _C1) + _C2))))
    spec = _Spec(body=body, reference=_exp8_ref)
    op = _dve_ops.DveOp('EXP8_CUBIC_ANT', spec, subdim=False, uops_sha={})
    row = max(_dve_ops._SUB_OPCODE_FOR_NAME.values()) + 1
    _dve_ops.OPS.append(op)
    _dve_ops.CUSTOM_DVE_SPECS[op.name] = op.spec
    _dve_ops._SUB_OPCODE_FOR_NAME[op.name] = row
    for ver in ('v3', 'v4'):
        compiled = _DveOpSpec(name=op.name, opcode=row,
                              uops=_dve_lower(spec, ver=ver),
                              rd1_en=_dve_has_src1(spec))
        op.uops_sha[ver] = compiled.sha(ver)
    return op


EXP8_OP = _register_exp8()

B, S, H, NH, HD = 2, 2048, 1024, 16, 64
P = 128
NCORES = 8
GSIZE = 4                    # replica-group size (cores per batch)
TQ = S // GSIZE              # tokens per core = 512
FT = H // P                  # feature tiles = 8
MT = TQ // P                 # token tiles per core = 4
EPS = 1e-5
RG = [[0, 1, 2, 3], [4, 5, 6, 7]]
KVT = 2 * P * TQ             # elems per head-pair block (K^T tile + V tile)
HT = 2                       # head-pairs per AllGather (4 AllGathers, mesh)
NAG = FT // HT               # number of AllGathers
KVC = HT * P                 # kv-projection chunk cols = 256
WC = 512                     # weight chunk (output cols per streamed tile)

f32 = mybir.dt.float32
bf16 = mybir.dt.bfloat16
AF = mybir.ActivationFunctionType
ALU = mybir.AluOpType


def build_kernel():
    nc = bacc.Bacc("TRN2", target_bir_lowering=False, debug=False,
                   num_devices=NCORES)

    def din(name, shape, dt=f32):
        return nc.dram_tensor(name, shape, dt, kind="ExternalInput").ap()

    # inputs (per-core values supplied via in_maps). Weights are host-side
    # re-laid-out as [P, nchunk, FT, WC] so each chunk is one contiguous
    # per-partition DMA.
    xtp = din("xtp", [P, FT * TQ], bf16)    # x slice ^T, p-major
    xnb = din("xnb", [TQ, H])               # x slice natural + bo pre-added
    wqp = din("wqp", [P, H // WC, FT, WC], bf16)
    wkp = din("wkp", [P, H // KVC, FT, KVC], bf16)  # scaled by 1/(8*sqrt(hd))
    wvp = din("wvp", [P, H // KVC, FT, KVC], bf16)
    wop = din("wop", [P, H // WC, FT, WC], bf16)
    w1p = din("w1p", [P, H // WC, FT, WC], bf16)
    w2p = din("w2p", [P, H // WC, FT, WC], bf16)
    bqp = din("bqp", [P, FT])               # bq as [part, tile]
    bkp = din("bkp", [P, FT])               # bk * 1/(8*sqrt(hd))
    b1p = din("b1p", [P, FT])               # b1 + W1 @ be1
    onesc = din("onesc", [P, MT], bf16)
    nri = din("nri", [P, 2], mybir.dt.int32)     # [1, -1] int scalars
    nrm = din("nrm", [P, MT], mybir.dt.int32)    # magic+1
    bvB = din("bvB", [P, H])                # broadcast rows
    b2B = din("b2B", [P, H])                # b2 + be1
    g1B = din("g1B", [P, H])
    g2B = din("g2B", [P, H])
    be2B = din("be2B", [P, H])
    y = nc.dram_tensor("y", [TQ, H], f32, kind="ExternalOutput").ap()

    bounce = [nc.dram_tensor(f"bounce{g}", [HT, KVT], bf16).ap()
              for g in range(NAG)]
    agout = [nc.dram_tensor(f"agout{g}", [GSIZE, HT, KVT], bf16).ap()
             for g in range(NAG)]

    with tile.TileContext(nc) as tc, ExitStack() as ctx:
        # ---------------- persistent pools ----------------
        const = ctx.enter_context(tc.tile_pool(name="const", bufs=1))
        acts = ctx.enter_context(tc.tile_pool(name="acts", bufs=1))
        wpool = ctx.enter_context(tc.tile_pool(name="w", bufs=3))

        # PE warm-up: ~40 back-to-back junk matmuls trip the HAM activity
        # window so real phase-A matmuls run at 2.4 GHz, while the first
        # DMAs stream in.
        with tc.tile_pool(name="wu", bufs=1) as wup, \
             tc.tile_pool(name="psW", bufs=1, space="PSUM") as psW:
            junk = wup.tile([P, 2 * P], bf16)
            nc.vector.memset(junk[:], 0.0)
            psw = psW.tile([P, 2 * P], f32)
            for _ in range(90):
                nc.tensor.matmul(psw[:], junk[:, 0:P], junk[:],
                                 start=True, stop=True)
            wsink = wup.tile([P, 1], f32)
            nc.vector.tensor_copy(wsink[:], psw[:, 0:1])

        # x^T load first — it gates the first projection matmuls
        xa = ctx.enter_context(tc.tile_pool(name="xa", bufs=1))
        xt_s = xa.tile([P, FT, TQ], bf16)
        nc.sync.dma_start(xt_s[:], xtp.rearrange("p (t n) -> p t n", n=TQ))

        # constants
        bq_s = const.tile([P, FT], f32)
        nc.sync.dma_start(bq_s[:], bqp)
        bk_s = const.tile([P, FT], f32)
        nc.sync.dma_start(bk_s[:], bkp)
        b1_s = const.tile([P, FT], f32)
        nc.sync.dma_start(b1_s[:], b1p)
        nri_s = const.tile([P, 2], mybir.dt.int32)
        nc.sync.dma_start(nri_s[:], nri)
        nrm_s = const.tile([P, MT], mybir.dt.int32)
        nc.sync.dma_start(nrm_s[:], nrm)
        bvB_s = const.tile([P, H], f32)
        nc.sync.dma_start(bvB_s[:], bvB)
        b2B_s = const.tile([P, H], f32)
        g1B_s = const.tile([P, H], f32)
        g2B_s = const.tile([P, H], f32)
        be2B_s = const.tile([P, H], f32)
        ones_s = const.tile([P, MT], bf16)
        nc.sync.dma_start(ones_s[:], onesc)
        eps_s = const.tile([P, 1], f32)
        nc.vector.memset(eps_s[:], EPS)
        warm_s = const.tile([P, 1], f32)
        nc.scalar.activation(warm_s[:], eps_s[:], AF.Exp)
        ident = const.tile([P, P], f32)
        make_identity(nc, ident)

        # resident activations (lifetimes span phases)
        xnb_s = acts.tile([P, MT, H], f32)       # natural x + bo
        qt_s = acts.tile([P, FT, TQ], bf16)      # Q^T
        ctxT_s = acts.tile([P, FT, TQ], bf16)    # attention ctx^T (normalized)
        ln1_s = acts.tile([P, MT, H], f32)       # LN1 out (gamma applied)
        ln1b2_s = acts.tile([P, MT, H], f32)     # ln1 + b2 (residual for fc2)
        ln1T_s = acts.tile([P, FT, TQ], bf16)    # LN1 transposed
        hT_s = acts.tile([P, FT, TQ], bf16)      # relu(fc1), transposed

        # ------------- phase A: K, V projections + 4 AllGathers ----------
        with tc.tile_pool(name="kv", bufs=1) as kvp, \
             tc.tile_pool(name="psA1", bufs=2, space="PSUM") as psA:
            kt_s = kvp.tile([P, FT, TQ], bf16)   # K^T (scaled), all 8 pairs
            v_s = kvp.tile([P, MT, H], bf16)     # V natural
            for g in range(NAG):    # AllGather group g: head-pairs 2g, 2g+1
                wk_c = wpool.tile([P, FT, KVC], bf16, tag="w")
                nc.sync.dma_start(wk_c[:], wkp[:, g])
                for mi in range(HT):            # K^T tiles (t = 2g+mi)
                    t = g * HT + mi
                    ps = psA.tile([P, TQ], f32, tag="psA")
                    for kt in range(FT):
                        nc.tensor.matmul(ps[:], wk_c[:, kt, ts(mi, P)],
                                         xt_s[:, kt, :],
                                         start=(kt == 0), stop=(kt == FT - 1))
                    nc.scalar.activation(kt_s[:, t, :], ps[:], AF.Identity,
                                         bias=bk_s[:, t:t + 1])
                wv_c = wpool.tile([P, FT, KVC], bf16, tag="w")
                nc.sync.dma_start(wv_c[:], wvp[:, g])
                for m in range(MT):             # V chunk (cols 256g..)
                    ps = psA.tile([P, KVC], f32, tag="psA")
                    for kt in range(FT):
                        nc.tensor.matmul(ps[:], xt_s[:, kt, ts(m, P)],
                                         wv_c[:, kt, :],
                                         start=(kt == 0), stop=(kt == FT - 1))
                    nc.vector.tensor_tensor(
                        out=v_s[:, m, ts(g, KVC)], in0=ps[:],
                        in1=bvB_s[:, ts(g, KVC)], op=ALU.add)
                # bounce + one AllGather per 2-pair group (mesh regime)
                nc.sync.dma_start(
                    bounce[g][:, 0:P * TQ].rearrange("t (p n) -> p t n", p=P),
                    kt_s[:, g * HT:(g + 1) * HT, :])
                for tl in range(HT):
                    nc.sync.dma_start(
                        bounce[g][tl, P * TQ:KVT]
                        .rearrange("(m p f) -> p m f", p=P, f=P),
                        v_s[:, :, ts(g * HT + tl, P)])
                nc.gpsimd.collective_compute(
                    "AllGather", ALU.bypass, replica_groups=RG,
                    ins=[bounce[g]], outs=[agout[g]])
            # Q^T projection overlaps with the AllGathers
            for g in range(2):
                wq_c = wpool.tile([P, FT, WC], bf16, tag="w")
                nc.sync.dma_start(wq_c[:], wqp[:, g])
                for mi in range(WC // P):
                    t = g * (WC // P) + mi
                    ps = psA.tile([P, TQ], f32, tag="psA")
                    for kt in range(FT):
                        nc.tensor.matmul(ps[:], wq_c[:, kt, ts(mi, P)],
                                         xt_s[:, kt, :],
                                         start=(kt == 0), stop=(kt == FT - 1))
                    nc.scalar.activation(qt_s[:, t, :], ps[:], AF.Identity,
                                         bias=bq_s[:, t:t + 1])

        # deferred constant loads (not needed until phases B-F)
        nc.gpsimd.dma_start(xnb_s[:], xnb.rearrange("(m p) f -> p m f", p=P))
        nc.gpsimd.dma_start(b2B_s[:], b2B)
        nc.gpsimd.dma_start(g1B_s[:], g1B)
        nc.gpsimd.dma_start(g2B_s[:], g2B)
        nc.gpsimd.dma_start(be2B_s[:], be2B)

        # ---------------- phase B: attention ----------------
        with tc.tile_pool(name="kvt", bufs=6) as kvt, \
             tc.tile_pool(name="esb", bufs=4) as esb, \
             tc.tile_pool(name="psS", bufs=2, space="PSUM") as psS, \
             tc.tile_pool(name="psC", bufs=2, space="PSUM") as psC, \
             tc.tile_pool(name="rec", bufs=1) as rec:
            for t in range(FT):          # head pair (2t, 2t+1)
                g, tl = t // HT, t % HT
                ps_c0 = psC.tile([P, TQ], f32, tag="c0")   # rows0-63 ctx, 64 sums
                ps_c1 = psC.tile([P, TQ], f32, tag="c1")
                first = True
                for rb in range(GSIZE):
                    ktile = kvt.tile([P, TQ], bf16, tag="k")
                    nc.sync.dma_start(
                        ktile[:],
                        agout[g][rb, tl, 0:P * TQ].rearrange("(p n) -> p n", p=P))
                    vbase = agout[g][rb, tl, P * TQ:KVT] \
                        .rearrange("(m p f) -> p m f", p=P, f=P)
                    vt0 = kvt.tile([P, MT, HD + 1], bf16, tag="v0")
                    nc.sync.dma_start(vt0[:, :, 0:HD], vbase[:, :, 0:HD])
                    nc.vector.tensor_copy(vt0[:, :, HD:HD + 1],
                                          ones_s.unsqueeze(2))
                    vt1 = kvt.tile([P, MT, HD + 1], bf16, tag="v1")
                    nc.sync.dma_start(vt1[:, :, 0:HD], vbase[:, :, HD:P])
                    nc.vector.tensor_copy(vt1[:, :, HD:HD + 1],
                                          ones_s.unsqueeze(2))
                    for sj in range(MT):
                        last = (rb == GSIZE - 1 and sj == MT - 1)
                        ps = psS.tile([P, 2, TQ], f32, tag="s")
                        nc.tensor.matmul(ps[:, 0, :],
                                         ktile[0:HD, ts(sj, P)],
                                         qt_s[0:HD, t, :],
                                         start=True, stop=True)
                        nc.tensor.matmul(ps[:, 1, :],
                                         ktile[HD:P, ts(sj, P)],
                                         qt_s[HD:P, t, :],
                                         start=True, stop=True)
                        e = esb.tile([P, 2, TQ], bf16, tag="e")
                        # head 2t: ScalarE table exp (scores arrive as s/8)
                        nc.scalar.activation(e[:, 0, :], ps[:, 0, :], AF.Exp,
                                             scale=8.0)
                        # head 2t+1: VectorE cubic^8 exp
                        nc.vector._custom_dve(
                            EXP8_OP, out=e[:, 1, :], in0=ps[:, 1, :],
                            s0=EXP_A, s1=EXP_B, imm2=EXP_C)
                        nc.tensor.matmul(ps_c0[0:HD + 1, :], vt0[:, sj, :],
                                         e[:, 0, :], start=first, stop=last)
                        nc.tensor.matmul(ps_c1[0:HD + 1, :], vt1[:, sj, :],
                                         e[:, 1, :], start=first, stop=last)
                        first = False
                # normalize: rows 0-63 / row 64
                sr0 = rec.tile([HD + 1, TQ], f32, tag="sr0")
                nc.vector.tensor_copy(sr0[HD:HD + 1, :], ps_c0[HD:HD + 1, :])
                rr0 = rec.tile([1, TQ], f32, tag="rr0")
                nc.gpsimd.dma_start(rr0[:], sr0[HD:HD + 1, :])
                nc.vector.reciprocal_approx_fast(rr0[:], rr0[:])
                rb0 = rec.tile([HD, TQ], f32, tag="rb0")
                nc.gpsimd.partition_broadcast(rb0[:], rr0[:])
                nc.vector.tensor_tensor(out=ctxT_s[0:HD, t, :], in0=ps_c0[0:HD, :],
                                        in1=rb0[:], op=ALU.mult)
                sr1 = rec.tile([HD + 1, TQ], f32, tag="sr1")
                nc.vector.tensor_copy(sr1[HD:HD + 1, :], ps_c1[HD:HD + 1, :])
                rr1 = rec.tile([1, TQ], f32, tag="rr1")
                nc.gpsimd.dma_start(rr1[:], sr1[HD:HD + 1, :])
                nc.vector.reciprocal_approx_fast(rr1[:], rr1[:])
                rb1 = rec.tile([HD, TQ], f32, tag="rb1")
                nc.gpsimd.partition_broadcast(rb1[:], rr1[:])
                c1t = rec.tile([HD, TQ], bf16, tag="c1t")
                nc.vector.tensor_tensor(out=c1t[:], in0=ps_c1[0:HD, :],
                                        in1=rb1[:], op=ALU.mult)
                nc.gpsimd.dma_start(ctxT_s[HD:P, t, :], c1t[:])

        # ---------------- LN helper ----------------
        def ln_stats(src_s, stat_pool):
            # src_s: [P, MT, H]; stats over free dim H. Returns (mv, rstd, nmr)
            mv = stat_pool.tile([P, MT, 2], f32, tag="mv")
            for m in range(MT):
                stats = stat_pool.tile([P, 2, 6], f32, tag="bst")
                for sg in range(2):
                    nc.vector.bn_stats(out=stats[:, sg, :],
                                       in_=src_s[:, m, ts(sg, H // 2)])
                nc.vector.bn_aggr(out=mv[:, m, :], in_=stats[:])
            # rstd = rsqrt(var + eps) via int-magic seed + 3 Newton steps
            ve = stat_pool.tile([P, MT], f32, tag="ve")
            nc.vector.tensor_scalar(out=ve[:], in0=mv[:, :, 1], scalar1=EPS,
                                    scalar2=None, op0=ALU.add)
            it = stat_pool.tile([P, MT], mybir.dt.int32, tag="it")
            nc.vector.tensor_scalar(out=it[:], in0=ve.bitcast(mybir.dt.int32),
                                    scalar1=nri_s[:, 0:1], scalar2=None,
                                    op0=ALU.logical_shift_right)
            nc.vector.tensor_scalar(out=it[:], in0=it[:], scalar1=nri_s[:, 1:2],
                                    scalar2=None, op0=ALU.bitwise_xor)
            nc.vector.tensor_tensor(out=it[:], in0=it[:], in1=nrm_s[:],
                                    op=ALU.add)
            rstd = it.bitcast(f32)
            nrt = stat_pool.tile([P, MT], f32, tag="nrt")
            for _ in range(3):
                nc.vector.tensor_tensor(out=nrt[:], in0=rstd, in1=rstd,
                                        op=ALU.mult)
                nc.vector.tensor_tensor(out=nrt[:], in0=nrt[:], in1=ve[:],
                                        op=ALU.mult)
                nc.vector.tensor_scalar(out=nrt[:], in0=nrt[:], scalar1=-0.5,
                                        scalar2=1.5, op0=ALU.mult, op1=ALU.add)
                nc.vector.tensor_tensor(out=rstd, in0=rstd, in1=nrt[:],
                                        op=ALU.mult)
            # nmr = -mu * rstd  (per-token bias for the fused affine apply)
            nmr = stat_pool.tile([P, MT], f32, tag="nmr")
            nc.vector.tensor_tensor(out=nmr[:], in0=mv[:, :, 0], in1=rstd,
                                    op=ALU.mult)
            nc.vector.tensor_scalar(out=nmr[:], in0=nmr[:], scalar1=-1.0,
                                    scalar2=None, op0=ALU.mult)
            return rstd, nmr

        from concourse.dve_ops import AFFINE_MUL_REDUCE

        # ---------------- phases C-F ----------------
        with tc.tile_pool(name="lnp", bufs=2) as lnp, \
             tc.tile_pool(name="wmm", bufs=1) as wmmpool, \
             tc.tile_pool(name="psA2", bufs=2, space="PSUM") as psA, \
             tc.tile_pool(name="pst", bufs=2, space="PSUM") as pst:
            # C: Wo + residual -> t1, then LN1 (fused gamma apply)
            t1_s = acts.tile([P, MT, H], f32, tag="tres")
            wo_cs = []
            for g in range(2):
                w_s = wmmpool.tile([P, FT, WC], bf16, tag=f"wmm{g}")
                nc.sync.dma_start(w_s[:], wop[:, g])
                wo_cs.append(w_s)
            for m in range(MT):
                for g in range(2):
                    ps = psA.tile([P, WC], f32, tag="psN")
                    for kt in range(FT):
                        nc.tensor.matmul(ps[:], ctxT_s[:, kt, ts(m, P)],
                                         wo_cs[g][:, kt, :],
                                         start=(kt == 0), stop=(kt == FT - 1))
                    nc.vector.tensor_tensor(
                        out=t1_s[:, m, ts(g, WC)], in0=ps[:],
                        in1=xnb_s[:, m, ts(g, WC)], op=ALU.add)
            rstd1, nmr1 = ln_stats(t1_s, lnp)
            for m in range(MT):
                nc.vector._custom_dve(
                    AFFINE_MUL_REDUCE, out=ln1_s[:, m, :], in0=t1_s[:, m, :],
                    in1=g1B_s[:], s0=rstd1[:, m:m + 1], s1=nmr1[:, m:m + 1])
                # residual for fc2 (b2 includes the folded be1)
                nc.gpsimd.tensor_tensor(out=ln1b2_s[:, m, :],
                                        in0=ln1_s[:, m, :],
                                        in1=b2B_s[:], op=ALU.add)

            # D: transpose ln1 -> ln1T (PE transpose, ScalarE eviction)
            for ft in range(FT):
                for m in range(MT):
                    pt = pst.tile([P, P], f32, tag="pt")
                    nc.tensor.transpose(pt[:], ln1_s[:, m, ts(ft, P)], ident[:])
                    nc.scalar.copy(ln1T_s[:, ft, ts(m, P)], pt[:])

            # E: fc1 + relu on ScalarE (transposed out)
            for g in range(2):
                w_s = wpool.tile([P, FT, WC], bf16, tag="w")
                nc.sync.dma_start(w_s[:], w1p[:, g])
                for mi in range(WC // P):
                    mt_i = g * (WC // P) + mi
                    ps = psA.tile([P, TQ], f32, tag="psT")
                    for kt in range(FT):
                        nc.tensor.matmul(ps[:], w_s[:, kt, ts(mi, P)],
                                         ln1T_s[:, kt, :],
                                         start=(kt == 0), stop=(kt == FT - 1))
                    nc.scalar.activation(hT_s[:, mt_i, :], ps[:], AF.Relu,
                                         bias=b1_s[:, mt_i:mt_i + 1])

            # F: fc2 + residual + LN2 + out
            t2_s = acts.tile([P, MT, H], f32, tag="tres")
            w2_cs = []
            for g in range(2):
                w_s = wmmpool.tile([P, FT, WC], bf16, tag=f"wmm{g}")
                nc.sync.dma_start(w_s[:], w2p[:, g])
                w2_cs.append(w_s)
            for m in range(MT):
                for g in range(2):
                    ps = psA.tile([P, WC], f32, tag="psN")
                    for kt in range(FT):
                        nc.tensor.matmul(ps[:], hT_s[:, kt, ts(m, P)],
                                         w2_cs[g][:, kt, :],
                                         start=(kt == 0), stop=(kt == FT - 1))
                    nc.vector.tensor_tensor(
                        out=t2_s[:, m, ts(g, WC)], in0=ps[:],
                        in1=ln1b2_s[:, m, ts(g, WC)], op=ALU.add)
            rstd2, nmr2 = ln_stats(t2_s, lnp)
            y_r = y.rearrange("(m p) f -> p m f", p=P)
            for m in range(MT):
                nc.vector._custom_dve(
                    AFFINE_MUL_REDUCE, out=t2_s[:, m, :], in0=t2_s[:, m, :],
                    in1=g2B_s[:], s0=rstd2[:, m:m + 1], s1=nmr2[:, m:m + 1])
                nc.vector.tensor_tensor(out=t2_s[:, m, :], in0=t2_s[:, m, :],
                                        in1=be2B_s[:], op=ALU.add)
                nc.sync.dma_start(y_r[:, m, :], t2_s[:, m, :])

    nc.compile()
    return nc


_NC_CACHE = {}


def _get_nc():
    if 'nc' not in _NC_CACHE:
        _NC_CACHE['nc'] = build_kernel()
    return _NC_CACHE['nc']


def _bf(a):
    return np.ascontiguousarray(np.asarray(a, np.float32)).astype(
        ml_dtypes.bfloat16)


def _wchunks(WT, wc=WC):
    # [H_in, H_out] -> [P, H//wc, FT, wc]: w[p, h, t, c] = WT[t*128+p, h*wc+c]
    w = np.asarray(WT, np.float32).reshape(FT, P, H // wc, wc)
    return _bf(np.ascontiguousarray(w.transpose(1, 2, 0, 3)))


def make_in_maps(x, Wq, bq, Wk, bk, Wv, bv, Wo, bo, W1, b1, W2, b2,
                 g1, be1, g2, be2):
    def pt(b):  # [H] -> [P, FT] partition-tiled
        return np.ascontiguousarray(np.asarray(b, np.float32).reshape(FT, P).T)

    def bc(v):  # [H] -> [P, H] broadcast
        return np.ascontiguousarray(
            np.broadcast_to(np.asarray(v, np.float32), (P, H)))

    scale = np.float32(1.0 / (8.0 * np.sqrt(HD)))
    W1f = np.asarray(W1, np.float32)
    b1f = np.asarray(b1, np.float32) + W1f @ np.asarray(be1, np.float32)
    b2f = np.asarray(b2, np.float32) + np.asarray(be1, np.float32)
    shared = {
        "wqp": _wchunks(np.asarray(Wq, np.float32).T),
        "wkp": _wchunks(np.asarray(Wk, np.float32).T * scale, KVC),
        "wvp": _wchunks(np.asarray(Wv, np.float32).T, KVC),
        "wop": _wchunks(np.asarray(Wo, np.float32).T),
        "w1p": _wchunks(W1f.T),
        "w2p": _wchunks(np.asarray(W2, np.float32).T),
        "bqp": pt(bq),
        "bkp": pt(np.asarray(bk, np.float32) * scale),
        "b1p": pt(b1f),
        "bvB": bc(bv), "b2B": bc(b2f),
        "g1B": bc(g1), "g2B": bc(g2), "be2B": bc(be2),
        "onesc": np.ones((P, MT), ml_dtypes.bfloat16),
        "nri": np.tile(np.array([[1, -1]], np.int32), (P, 1)),
        "nrm": np.full((P, MT), 0x5f3759df + 1, np.int32),
    }
    in_maps = []
    for c in range(NCORES):
        b, sl = c // GSIZE, (c % GSIZE) * TQ
        xs = np.asarray(x[b, sl:sl + TQ, :], np.float32)
        m = dict(shared)
        xT = _bf(xs.T)  # [H, TQ]
        m["xtp"] = np.ascontiguousarray(
            xT.reshape(FT, P, TQ).transpose(1, 0, 2).reshape(P, FT * TQ))
        m["xnb"] = np.ascontiguousarray(xs + np.asarray(bo, np.float32))
        in_maps.append(m)
    return in_maps


def kernel(x, Wq, bq, Wk, bk, Wv, bv, Wo, bo, W1, b1, W2, b2,
           g1, be1, g2, be2):
    x = np.asarray(x)
    nc = _get_nc()
    in_maps = make_in_maps(x, Wq, bq, Wk, bk, Wv, bv, Wo, bo,
                           W1, b1, W2, b2, g1, be1, g2, be2)
    res = run_bass_kernel_spmd(nc, in_maps, list(range(NCORES)))
    out = np.empty((B, S, H), np.float32)
    for c in range(NCORES):
        b, sl = c // GSIZE, (c % GSIZE) * TQ
        out[b, sl:sl + TQ, :] = res.results[c]["y"]
    return out
